# revision 1
# baseline (speedup 1.0000x reference)
"""MoE-LoRA layer kernel for Trainium2 (8 NeuronCores, data-parallel over tokens).

Computation (per reference):
  out = x @ W_base.T + b_base + scaling * sum_e combine[:,e] * (x @ A_e.T) @ B_e.T
  combine = renormalized top-2 softmax of router logits (= softmax over top-2 logits).

Sharding: 8192 tokens -> 1024 per core; all weights replicated. Everything
is laid out host-side so device DMAs are contiguous:
  xt[p, kt, t]      = x[t, kt*128+p]           (x transposed, k-tiled)
  wt[ot, p, kt, o]  = W_base[ot*128+o, kt*128+p]  (W_base.T in per-o-tile slabs)
  at[p, kt, er]     = A_all[er, kt*128+p]
  bt[er, o]         = B_stack[e, o, r],  er = e*16+r
  rt[p, kt, e]      = W_router[e, kt*128+p]
  bias2[p, ot]      = b_base[ot*128+p]
Output: outt[ot, p, t] = out[t, ot*128+p].

Matmuls run as float32r (full fp32 input bits; 1 cycle/row at free-dim>=256).
"""

import sys
import numpy as np
from contextlib import ExitStack

try:
    import concourse.bass as bass
except ImportError:
    sys.path.insert(0, "/opt/trn_rl_repo")
    import concourse.bass as bass

import concourse.tile as tile
from concourse import bacc
from concourse import mybir
from concourse.bass import ts
from concourse.bass_utils import run_bass_kernel_spmd

F32 = mybir.dt.float32
F32R = mybir.dt.float32r
ALU = mybir.AluOpType
ACTF = mybir.ActivationFunctionType
AX = mybir.AxisListType

N_CORES = 8
D_IN = 4096
D_OUT = 4096
RANK = 16
NUM_EXPERTS = 8
ER = NUM_EXPERTS * RANK  # 128
TOP_K = 2
SCALING = 32.0 / RANK  # 2.0


def build_nc(T=1024, KT=32, OT=32):
    """Build the per-core Bass kernel. T tokens, KT k-tiles (d_in=128*KT),
    OT out-tiles (d_out=128*OT). T must be a multiple of 512."""
    TH = T // 512  # token halves for 512-wide matmuls
    TS = T // 128  # token subtiles for router/softmax
    nc = bacc.Bacc(None, target_bir_lowering=False, dynamic_dma_scratch_size=1024)

    xt = nc.dram_tensor("xt", [128, KT, T], F32R, kind="ExternalInput")
    wt = nc.dram_tensor("wt", [OT, 128, KT, 128], F32R, kind="ExternalInput")
    at = nc.dram_tensor("at", [128, KT, ER], F32R, kind="ExternalInput")
    bt = nc.dram_tensor("bt", [ER, 128 * OT], F32R, kind="ExternalInput")
    rt = nc.dram_tensor("rt", [128, KT, NUM_EXPERTS], F32R, kind="ExternalInput")
    bias2 = nc.dram_tensor("bias2", [128, OT], F32, kind="ExternalInput")
    id2 = nc.dram_tensor("id2", [128, 128], F32, kind="ExternalInput")
    expand = nc.dram_tensor("expand", [NUM_EXPERTS, ER], F32, kind="ExternalInput")
    outt = nc.dram_tensor("outt", [OT, 128, T], F32, kind="ExternalOutput")

    with tile.TileContext(nc) as tc, ExitStack() as ctx:
        const = ctx.enter_context(tc.tile_pool(name="const", bufs=1))
        xpool = ctx.enter_context(tc.tile_pool(name="xp", bufs=1))
        wpool = ctx.enter_context(tc.tile_pool(name="wp", bufs=4))
        btp = ctx.enter_context(tc.tile_pool(name="btp", bufs=2))
        hpool = ctx.enter_context(tc.tile_pool(name="hp", bufs=1))
        smt = ctx.enter_context(tc.tile_pool(name="smt", bufs=1))
        opool = ctx.enter_context(tc.tile_pool(name="op", bufs=2))
        pmain = ctx.enter_context(
            tc.tile_pool(name="pmain", bufs=max(3 * TH, 4), space="PSUM")
        )
        psmall = ctx.enter_context(tc.tile_pool(name="psm", bufs=2, space="PSUM"))
        E = NUM_EXPERTS

        # ---- window DMAs on the sync HWDGE queue, interleaved so the first
        # k-tiles land fast; x alternates sync/scalar queues for bandwidth ----
        rt_s = const.tile([128, KT, E], F32R)
        nc.sync.dma_start(rt_s, rt[:])
        at_s = wpool.tile([128, KT, 128], F32R, tag="w")
        w0_s = wpool.tile([128, KT, 128], F32R, tag="w")
        nsw = min(8, KT)
        wpc = KT // nsw
        x_s = xpool.tile([128, KT, T], F32R)

        def xdma(kt, eng):
            if kt < 2 and KT >= 32:
                for th in range(TH):
                    eng.dma_start(x_s[:, kt, ts(th, 512)], xt[:, kt, ts(th, 512)])
            else:
                eng.dma_start(x_s[:, kt, :], xt[:, kt, :])

        id_s = bias_s = exp_s = b0_s = None

        def emit_consts():
            nonlocal id_s, bias_s, exp_s, b0_s
            id_s = const.tile([128, 128], F32)
            nc.sync.dma_start(id_s, id2[:])
            bias_s = const.tile([128, OT], F32)
            nc.sync.dma_start(bias_s, bias2[:])
            exp_s = const.tile([E, ER], F32)
            nc.sync.dma_start(exp_s, expand[:])
            b0_s = const.tile([ER, 128], F32R)
            nc.scalar.dma_start(b0_s, bt[:, 0:128])


        pre = nsw
        for q in range(pre):
            if q == 0 and wpc > 1:
                # only kt=0's weight slivers ahead of the first x chunks
                nc.sync.dma_start(at_s[:, 0:1, :], at[:, 0:1, :])
                nc.sync.dma_start(w0_s[:, 0:1, :], wt[0, :, 0:1, :])
                if KT >= 32:
                    for th in range(TH):
                        nc.sync.dma_start(
                            x_s[:, 0, ts(th, 512)], xt[:, 0, ts(th, 512)]
                        )
                nc.sync.dma_start(at_s[:, 1:wpc, :], at[:, 1:wpc, :])
                nc.sync.dma_start(w0_s[:, 1:wpc, :], wt[0, :, 1:wpc, :])
            else:
                nc.sync.dma_start(at_s[:, ts(q, wpc), :], at[:, ts(q, wpc), :])
                nc.sync.dma_start(w0_s[:, ts(q, wpc), :], wt[0, :, ts(q, wpc), :])
            if q == 0 or KT < 32:
                for kt in range(q * wpc, (q + 1) * wpc):
                    if q == 0 and kt == 0 and wpc > 1 and KT >= 32:
                        continue
                    xdma(kt, nc.sync)
            else:
                # 2-ktile (1MB) chunks: fewer descriptors, higher sustained rate
                for k0 in range(q * wpc, (q + 1) * wpc, 2):
                    nc.sync.dma_start(x_s[:, k0 : k0 + 2, :], xt[:, k0 : k0 + 2, :])
            if q == 5:
                emit_consts()
        for q in range(pre, nsw):
            nc.sync.dma_start(at_s[:, ts(q, wpc), :], at[:, ts(q, wpc), :])
            nc.sync.dma_start(w0_s[:, ts(q, wpc), :], wt[0, :, ts(q, wpc), :])
        if id_s is None:
            emit_consts()

        # ---- window: A-proj + router + base(ot=0) share the x stream ----
        ph = [pmain.tile([128, 512], F32, tag="pm", name=f"ph{i}") for i in range(TH)]
        plT = [pmain.tile([E, 512], F32, tag="pm", name=f"plT{i}") for i in range(TH)]
        po0 = [pmain.tile([128, 512], F32, tag="pm", name=f"po0{i}") for i in range(TH)]
        for kt in range(KT):
            st, sp = kt == 0, kt == KT - 1
            xcs = [x_s[:, kt, ts(th, 512)] for th in range(TH)]
            for th in range(TH):
                nc.tensor.matmul(ph[th], at_s[:, kt, :], xcs[th], start=st, stop=sp)
            for th in range(TH):
                nc.tensor.matmul(plT[th], rt_s[:, kt, :], xcs[th], start=st, stop=sp)
            for th in range(TH):
                nc.tensor.matmul(po0[th], w0_s[:, kt, :], xcs[th], start=st, stop=False)

        def load_w(ot):
            w_s = wpool.tile([128, KT, 128], F32R, tag="w")
            nsl = min(4, KT)
            for q in range(nsl):
                nc.sync.dma_start(
                    w_s[:, ts(q, KT // nsl), :], wt[ot, :, ts(q, KT // nsl), :]
                )
            b_sl = btp.tile([ER, 128], F32R)
            nc.sync.dma_start(b_sl, bt[:, ts(ot, 128)])
            return w_s, b_sl

        h_s = hpool.tile([128, T], F32R)
        hw_r = h_s  # weighted in place; f32r typing for the B matmul
        h_f = h_s.bitcast(F32)  # f32 read view for the DVE weighting
        lT = hpool.tile([E, T], F32)
        for th in range(TH):
            nc.vector.tensor_copy(h_s[:, ts(th, 512)], ph[th])
            nc.vector.tensor_copy(lT[:, ts(th, 512)], plT[th])

        def emit_base(ot, w_s):
            # kt outer / th inner: consecutive matmuls share the stationary
            # weight tile, letting the weight load amortize over the pair
            pos = [
                pmain.tile([128, 512], F32, tag="pm", name=f"po_{ot}_{th}")
                for th in range(TH)
            ]
            for kt in range(KT):
                for th in range(TH):
                    nc.tensor.matmul(
                        pos[th],
                        w_s[:, kt, :],
                        x_s[:, kt, ts(th, 512)],
                        start=(kt == 0),
                        stop=False,
                    )
            return pos

        def emit_tail(ot, pos, b_sl):
            for th in range(TH):
                nc.tensor.matmul(
                    pos[th], b_sl, hw_r[:, ts(th, 512)], start=False, stop=True
                )
                o_t = opool.tile([128, 512], F32, tag="o_t", name=f"ot_{ot}_{th}")
                nc.scalar.activation(
                    o_t, pos[th], ACTF.Identity, bias=bias_s[:, ot : ot + 1]
                )
                nc.sync.dma_start(outt[ot, :, ts(th, 512)], o_t)

        first = min(1, OT - 1)
        w1, b1 = load_w(first)
        pos1 = emit_base(first, w1)

        # ---- softmax/top-2 (hides behind ot=1 base matmuls) ----
        for s_i in range(TS):
            ptl = psmall.tile([128, E], F32, tag="ps", name="ptl")
            nc.tensor.transpose(ptl, lT[:, ts(s_i, 128)], id_s[:E, :E])
            l = smt.tile([128, E], F32)
            nc.vector.tensor_copy(l, ptl)
            m1 = smt.tile([128, 1], F32)
            nc.vector.reduce_max(m1, l, axis=AX.X)
            lm = smt.tile([128, E], F32)  # logits - max  (<= 0, ==0 at argmax)
            nc.vector.tensor_scalar(lm, l, m1, None, op0=ALU.subtract)
            isz = smt.tile([128, E], F32)
            nc.vector.tensor_scalar(isz, lm, 0.0, None, op0=ALU.is_equal)
            pen = smt.tile([128, E], F32)
            nc.vector.tensor_scalar(pen, isz, -1e30, None, op0=ALU.mult)
            msk = smt.tile([128, E], F32)
            nc.vector.tensor_tensor(msk, lm, pen, op=ALU.add)
            m2 = smt.tile([128, 1], F32)  # second max, relative to m1
            nc.vector.reduce_max(m2, msk, axis=AX.X)
            e_t = smt.tile([128, E], F32)
            nc.scalar.activation(e_t, lm, ACTF.Exp)
            e2 = smt.tile([128, 1], F32)
            nc.scalar.activation(e2, m2, ACTF.Exp)
            den = smt.tile([128, 1], F32)
            nc.vector.tensor_scalar(den, e2, 1.0, None, op0=ALU.add)
            inv = smt.tile([128, 1], F32)
            nc.vector.reciprocal(inv, den)
            ge = smt.tile([128, E], F32)  # top-2 membership mask
            nc.vector.tensor_scalar(ge, lm, m2, None, op0=ALU.is_ge)
            cmb = smt.tile([128, E], F32)
            nc.vector.tensor_tensor(cmb, e_t, ge, op=ALU.mult)
            cmb2 = smt.tile([128, E], F32)
            nc.vector.tensor_scalar(cmb2, cmb, inv, None, op0=ALU.mult)
            pt = psmall.tile([E, 128], F32, tag="ps", name="pt")
            nc.tensor.transpose(pt, cmb2, id_s)
            ct = smt.tile([E, 128], F32)
            nc.vector.tensor_copy(ct, pt)
            pc = psmall.tile([128, 128], F32, tag="ps", name="pc")
            nc.tensor.matmul(pc, exp_s, ct, start=True, stop=True)
            nc.vector.tensor_tensor(
                hw_r[:, ts(s_i, 128)], h_f[:, ts(s_i, 128)], pc, op=ALU.mult
            )

        emit_tail(first, pos1, b1)

        # ---- ot=0 LoRA term accumulated into the held PSUM group ----
        for th in range(TH):
            nc.tensor.matmul(
                po0[th], b0_s, hw_r[:, ts(th, 512)], start=False, stop=True
            )
            o_t = opool.tile([128, 512], F32, name=f"oo0_{th}", tag="o_t")
            nc.scalar.activation(o_t, po0[th], ACTF.Identity, bias=bias_s[:, 0:1])
            nc.sync.dma_start(outt[0, :, ts(th, 512)], o_t)

        # ---- remaining o-tiles, fused B-projection ----
        for ot in range(2, OT):
            w_s, b_sl = load_w(ot)
            pos = emit_base(ot, w_s)
            emit_tail(ot, pos, b_sl)

    nc.compile()
    return nc


def prep_shared(W_base, b_base, W_router, A_stack, B_stack, KT=32, OT=32):
    """Host-side layout prep for the replicated weights."""
    D = KT * 128
    O = OT * 128
    W_base = np.asarray(W_base, dtype=np.float32)
    wt = np.ascontiguousarray(W_base.reshape(OT, 128, KT, 128).transpose(0, 3, 2, 1))
    A_all = np.asarray(A_stack, dtype=np.float32).reshape(ER, D)
    at = np.ascontiguousarray(A_all.reshape(ER, KT, 128).transpose(2, 1, 0))
    bt = np.ascontiguousarray(
        np.asarray(B_stack, dtype=np.float32).transpose(0, 2, 1).reshape(ER, O)
    )
    rtT = np.asarray(W_router, dtype=np.float32).T  # [D, E]
    rt = np.ascontiguousarray(rtT.reshape(KT, 128, NUM_EXPERTS).transpose(1, 0, 2))
    bias2 = np.ascontiguousarray(np.asarray(b_base, dtype=np.float32).reshape(OT, 128).T)
    id2 = np.eye(128, dtype=np.float32)
    expand = np.repeat(
        np.eye(NUM_EXPERTS, dtype=np.float32) * np.float32(SCALING), RANK, axis=1
    )
    return dict(wt=wt, at=at, bt=bt, rt=rt, bias2=bias2, id2=id2, expand=expand)


_NC_CACHE = {}


def _get_nc(T, KT, OT):
    key = (T, KT, OT)
    if key not in _NC_CACHE:
        _NC_CACHE[key] = build_nc(T, KT, OT)
    return _NC_CACHE[key]


def kernel(x, W_base, b_base, W_router, A_stack, B_stack):
    x = np.asarray(x, dtype=np.float32)
    orig_shape = x.shape
    xf = x.reshape(-1, D_IN)
    N = xf.shape[0]
    T = N // N_CORES
    KT = D_IN // 128
    OT = D_OUT // 128

    nc = _get_nc(T, KT, OT)
    shared = prep_shared(W_base, b_base, W_router, A_stack, B_stack, KT, OT)

    in_maps = []
    for c in range(N_CORES):
        x_c = xf[c * T : (c + 1) * T]  # [T, D]
        xt = np.ascontiguousarray(x_c.reshape(T, KT, 128).transpose(2, 1, 0))
        m = dict(shared)
        m["xt"] = xt
        in_maps.append(m)

    res = run_bass_kernel_spmd(nc, in_maps, core_ids=list(range(N_CORES)))
    out = np.empty((N, D_OUT), dtype=np.float32)
    for c in range(N_CORES):
        outt = res.results[c]["outt"]  # [OT, 128, T]
        out[c * T : (c + 1) * T] = outt.transpose(2, 0, 1).reshape(T, D_OUT)
    return out.reshape(orig_shape[:-1] + (D_OUT,))



# revision 2
# speedup vs baseline: 1.0756x; 1.0756x over previous
"""MoE-LoRA layer kernel for Trainium2 (8 NeuronCores, data-parallel over tokens).

Computation (per reference):
  out = x @ W_base.T + b_base + scaling * sum_e combine[:,e] * (x @ A_e.T) @ B_e.T
  combine = renormalized top-2 softmax of router logits (= softmax over top-2 logits).

Sharding: 8192 tokens -> 1024 per core; all weights replicated. Everything
is laid out host-side so device DMAs are contiguous:
  xt[p, kt, t]      = x[t, kt*128+p]           (x transposed, k-tiled)
  wt[ot, p, kt, o]  = W_base[ot*128+o, kt*128+p]  (W_base.T in per-o-tile slabs)
  at[p, kt, er]     = A_all[er, kt*128+p]
  bt[er, o]         = B_stack[e, o, r],  er = e*16+r
  rt[p, kt, e]      = W_router[e, kt*128+p]
  bias2[p, ot]      = b_base[ot*128+p]
Output: outt[ot, p, t] = out[t, ot*128+p].

All GEMM operands are bf16 (PSUM accumulates fp32): same 1 cycle/row PE rate
as float32r but half the HBM/SBUF traffic and a 2-byte LDWEIGHTS, which lets
back-to-back matmuls reach the stream-rate cadence.
"""

import sys
import numpy as np
import ml_dtypes
from contextlib import ExitStack

try:
    import concourse.bass as bass
except ImportError:
    sys.path.insert(0, "/opt/trn_rl_repo")
    import concourse.bass as bass

import concourse.tile as tile
from concourse import bacc
from concourse import mybir
from concourse.bass import ts
from concourse.bass_utils import run_bass_kernel_spmd

F32 = mybir.dt.float32
BF16 = mybir.dt.bfloat16
ALU = mybir.AluOpType
ACTF = mybir.ActivationFunctionType
AX = mybir.AxisListType
NPBF16 = ml_dtypes.bfloat16

N_CORES = 8
D_IN = 4096
D_OUT = 4096
RANK = 16
NUM_EXPERTS = 8
ER = NUM_EXPERTS * RANK  # 128
TOP_K = 2
SCALING = 32.0 / RANK  # 2.0


def build_nc(T=1024, KT=32, OT=32):
    """Build the per-core Bass kernel. T tokens, KT k-tiles (d_in=128*KT),
    OT out-tiles (d_out=128*OT). T must be a multiple of 512."""
    TH = T // 512  # token halves for 512-wide matmuls
    TS = T // 128  # token subtiles for router/softmax
    nc = bacc.Bacc(None, target_bir_lowering=False, dynamic_dma_scratch_size=1024)

    xt = nc.dram_tensor("xt", [128, KT, T], BF16, kind="ExternalInput")
    wt = nc.dram_tensor("wt", [OT, 128, KT, 128], BF16, kind="ExternalInput")
    at = nc.dram_tensor("at", [128, KT, ER], BF16, kind="ExternalInput")
    bt = nc.dram_tensor("bt", [ER, 128 * OT], BF16, kind="ExternalInput")
    rt = nc.dram_tensor("rt", [128, KT, NUM_EXPERTS], BF16, kind="ExternalInput")
    bias2 = nc.dram_tensor("bias2", [128, OT], F32, kind="ExternalInput")
    id2 = nc.dram_tensor("id2", [128, 128], F32, kind="ExternalInput")
    expand = nc.dram_tensor("expand", [NUM_EXPERTS, ER], BF16, kind="ExternalInput")
    outt = nc.dram_tensor("outt", [OT, 128, T], BF16, kind="ExternalOutput")

    with tile.TileContext(nc) as tc, ExitStack() as ctx:
        const = ctx.enter_context(tc.tile_pool(name="const", bufs=1))
        xpool = ctx.enter_context(tc.tile_pool(name="xp", bufs=1))
        wpool = ctx.enter_context(tc.tile_pool(name="wp", bufs=4))
        btp = ctx.enter_context(tc.tile_pool(name="btp", bufs=2))
        hpool = ctx.enter_context(tc.tile_pool(name="hp", bufs=1))
        smt = ctx.enter_context(tc.tile_pool(name="smt", bufs=1))
        opool = ctx.enter_context(tc.tile_pool(name="op", bufs=2))
        pmain = ctx.enter_context(
            tc.tile_pool(name="pmain", bufs=max(3 * TH, 4), space="PSUM")
        )
        psmall = ctx.enter_context(tc.tile_pool(name="psm", bufs=2, space="PSUM"))
        E = NUM_EXPERTS

        # ---- window DMAs on the sync HWDGE queue, interleaved so the first
        # k-tiles land fast ----
        rt_s = const.tile([128, KT, E], BF16)
        nc.sync.dma_start(rt_s, rt[:])
        at_s = wpool.tile([128, KT, 128], BF16, tag="w")
        w0_s = wpool.tile([128, KT, 128], BF16, tag="w")
        nsw = min(8, KT)
        wpc = KT // nsw
        x_s = xpool.tile([128, KT, T], BF16)

        def xdma(kt, eng):
            if kt < 2 and KT >= 32:
                for th in range(TH):
                    eng.dma_start(x_s[:, kt, ts(th, 512)], xt[:, kt, ts(th, 512)])
            else:
                eng.dma_start(x_s[:, kt, :], xt[:, kt, :])

        id_s = bias_s = exp_s = b0_s = None

        def emit_consts():
            nonlocal id_s, bias_s, exp_s, b0_s
            id_s = const.tile([128, 128], F32)
            nc.sync.dma_start(id_s, id2[:])
            bias_s = const.tile([128, OT], F32)
            nc.sync.dma_start(bias_s, bias2[:])
            exp_s = const.tile([E, ER], BF16)
            nc.sync.dma_start(exp_s, expand[:])
            b0_s = const.tile([ER, 128], BF16)
            nc.scalar.dma_start(b0_s, bt[:, 0:128])


        pre = nsw
        for q in range(pre):
            if q == 0 and wpc > 1:
                # only kt=0's weight slivers ahead of the first x chunks
                nc.sync.dma_start(at_s[:, 0:1, :], at[:, 0:1, :])
                nc.sync.dma_start(w0_s[:, 0:1, :], wt[0, :, 0:1, :])
                if KT >= 32:
                    for th in range(TH):
                        nc.sync.dma_start(
                            x_s[:, 0, ts(th, 512)], xt[:, 0, ts(th, 512)]
                        )
                nc.sync.dma_start(at_s[:, 1:wpc, :], at[:, 1:wpc, :])
                nc.sync.dma_start(w0_s[:, 1:wpc, :], wt[0, :, 1:wpc, :])
            else:
                nc.sync.dma_start(at_s[:, ts(q, wpc), :], at[:, ts(q, wpc), :])
                nc.sync.dma_start(w0_s[:, ts(q, wpc), :], wt[0, :, ts(q, wpc), :])
            if q == 0 or KT < 32:
                for kt in range(q * wpc, (q + 1) * wpc):
                    if q == 0 and kt == 0 and wpc > 1 and KT >= 32:
                        continue
                    xdma(kt, nc.sync)
            else:
                # 2-ktile (0.5MB) chunks: fewer descriptors, higher sustained rate
                for k0 in range(q * wpc, (q + 1) * wpc, 2):
                    nc.sync.dma_start(x_s[:, k0 : k0 + 2, :], xt[:, k0 : k0 + 2, :])
            if q == 5:
                emit_consts()
        for q in range(pre, nsw):
            nc.sync.dma_start(at_s[:, ts(q, wpc), :], at[:, ts(q, wpc), :])
            nc.sync.dma_start(w0_s[:, ts(q, wpc), :], wt[0, :, ts(q, wpc), :])
        if id_s is None:
            emit_consts()

        # ---- window: A-proj + router + base(ot=0) share the x stream ----
        ph = [pmain.tile([128, 512], F32, tag="pm", name=f"ph{i}") for i in range(TH)]
        plT = [pmain.tile([E, 512], F32, tag="pm", name=f"plT{i}") for i in range(TH)]
        po0 = [pmain.tile([128, 512], F32, tag="pm", name=f"po0{i}") for i in range(TH)]
        for kt in range(KT):
            st, sp = kt == 0, kt == KT - 1
            xcs = [x_s[:, kt, ts(th, 512)] for th in range(TH)]
            for th in range(TH):
                nc.tensor.matmul(ph[th], at_s[:, kt, :], xcs[th], start=st, stop=sp)
            for th in range(TH):
                nc.tensor.matmul(plT[th], rt_s[:, kt, :], xcs[th], start=st, stop=sp)
            for th in range(TH):
                nc.tensor.matmul(po0[th], w0_s[:, kt, :], xcs[th], start=st, stop=False)

        def load_w(ot):
            w_s = wpool.tile([128, KT, 128], BF16, tag="w")
            nsl = min(4, KT)
            for q in range(nsl):
                nc.sync.dma_start(
                    w_s[:, ts(q, KT // nsl), :], wt[ot, :, ts(q, KT // nsl), :]
                )
            b_sl = btp.tile([ER, 128], BF16)
            nc.sync.dma_start(b_sl, bt[:, ts(ot, 128)])
            return w_s, b_sl

        h_s = hpool.tile([128, T], BF16)
        hw_r = h_s  # weighted in place; rhs of the B matmuls
        lT = hpool.tile([E, T], F32)
        for th in range(TH):
            nc.vector.tensor_copy(h_s[:, ts(th, 512)], ph[th])
            nc.vector.tensor_copy(lT[:, ts(th, 512)], plT[th])

        def emit_base(ot, w_s):
            # kt outer / th inner: consecutive matmuls share the stationary
            # weight tile, letting the weight load amortize over the pair
            pos = [
                pmain.tile([128, 512], F32, tag="pm", name=f"po_{ot}_{th}")
                for th in range(TH)
            ]
            for kt in range(KT):
                for th in range(TH):
                    nc.tensor.matmul(
                        pos[th],
                        w_s[:, kt, :],
                        x_s[:, kt, ts(th, 512)],
                        start=(kt == 0),
                        stop=False,
                    )
            return pos

        def emit_tail(ot, pos, b_sl):
            for th in range(TH):
                nc.tensor.matmul(
                    pos[th], b_sl, hw_r[:, ts(th, 512)], start=False, stop=True
                )
                o_t = opool.tile([128, 512], BF16, tag="o_t", name=f"ot_{ot}_{th}")
                nc.scalar.activation(
                    o_t, pos[th], ACTF.Identity, bias=bias_s[:, ot : ot + 1]
                )
                nc.sync.dma_start(outt[ot, :, ts(th, 512)], o_t)

        first = min(1, OT - 1)
        w1, b1 = load_w(first)
        pos1 = emit_base(first, w1)

        # ---- softmax/top-2 (hides behind ot=1 base matmuls) ----
        for s_i in range(TS):
            ptl = psmall.tile([128, E], F32, tag="ps", name="ptl")
            nc.tensor.transpose(ptl, lT[:, ts(s_i, 128)], id_s[:E, :E])
            l = smt.tile([128, E], F32)
            nc.vector.tensor_copy(l, ptl)
            m1 = smt.tile([128, 1], F32)
            nc.vector.reduce_max(m1, l, axis=AX.X)
            lm = smt.tile([128, E], F32)  # logits - max  (<= 0, ==0 at argmax)
            nc.vector.tensor_scalar(lm, l, m1, None, op0=ALU.subtract)
            isz = smt.tile([128, E], F32)
            nc.vector.tensor_scalar(isz, lm, 0.0, None, op0=ALU.is_equal)
            pen = smt.tile([128, E], F32)
            nc.vector.tensor_scalar(pen, isz, -1e30, None, op0=ALU.mult)
            msk = smt.tile([128, E], F32)
            nc.vector.tensor_tensor(msk, lm, pen, op=ALU.add)
            m2 = smt.tile([128, 1], F32)  # second max, relative to m1
            nc.vector.reduce_max(m2, msk, axis=AX.X)
            e_t = smt.tile([128, E], F32)
            nc.scalar.activation(e_t, lm, ACTF.Exp)
            e2 = smt.tile([128, 1], F32)
            nc.scalar.activation(e2, m2, ACTF.Exp)
            den = smt.tile([128, 1], F32)
            nc.vector.tensor_scalar(den, e2, 1.0, None, op0=ALU.add)
            inv = smt.tile([128, 1], F32)
            nc.vector.reciprocal(inv, den)
            ge = smt.tile([128, E], F32)  # top-2 membership mask
            nc.vector.tensor_scalar(ge, lm, m2, None, op0=ALU.is_ge)
            cmb = smt.tile([128, E], F32)
            nc.vector.tensor_tensor(cmb, e_t, ge, op=ALU.mult)
            cmb2 = smt.tile([128, E], F32)
            nc.vector.tensor_scalar(cmb2, cmb, inv, None, op0=ALU.mult)
            pt = psmall.tile([E, 128], F32, tag="ps", name="pt")
            nc.tensor.transpose(pt, cmb2, id_s)
            ct = smt.tile([E, 128], BF16)
            nc.vector.tensor_copy(ct, pt)
            pc = psmall.tile([128, 128], F32, tag="ps", name="pc")
            nc.tensor.matmul(pc, exp_s, ct, start=True, stop=True)
            nc.vector.tensor_tensor(
                hw_r[:, ts(s_i, 128)], h_s[:, ts(s_i, 128)], pc, op=ALU.mult
            )

        emit_tail(first, pos1, b1)

        # ---- ot=0 LoRA term accumulated into the held PSUM group ----
        for th in range(TH):
            nc.tensor.matmul(
                po0[th], b0_s, hw_r[:, ts(th, 512)], start=False, stop=True
            )
            o_t = opool.tile([128, 512], BF16, name=f"oo0_{th}", tag="o_t")
            nc.scalar.activation(o_t, po0[th], ACTF.Identity, bias=bias_s[:, 0:1])
            nc.sync.dma_start(outt[0, :, ts(th, 512)], o_t)

        # ---- remaining o-tiles, fused B-projection ----
        for ot in range(2, OT):
            w_s, b_sl = load_w(ot)
            pos = emit_base(ot, w_s)
            emit_tail(ot, pos, b_sl)

    nc.compile()
    return nc


def prep_shared(W_base, b_base, W_router, A_stack, B_stack, KT=32, OT=32):
    """Host-side layout prep for the replicated weights."""
    D = KT * 128
    O = OT * 128
    W_base = np.asarray(W_base, dtype=np.float32)
    wt = np.ascontiguousarray(
        W_base.reshape(OT, 128, KT, 128).transpose(0, 3, 2, 1)
    ).astype(NPBF16)
    A_all = np.asarray(A_stack, dtype=np.float32).reshape(ER, D)
    at = np.ascontiguousarray(A_all.reshape(ER, KT, 128).transpose(2, 1, 0)).astype(
        NPBF16
    )
    bt = np.ascontiguousarray(
        np.asarray(B_stack, dtype=np.float32).transpose(0, 2, 1).reshape(ER, O)
    ).astype(NPBF16)
    rtT = np.asarray(W_router, dtype=np.float32).T  # [D, E]
    rt = np.ascontiguousarray(
        rtT.reshape(KT, 128, NUM_EXPERTS).transpose(1, 0, 2)
    ).astype(NPBF16)
    bias2 = np.ascontiguousarray(np.asarray(b_base, dtype=np.float32).reshape(OT, 128).T)
    id2 = np.eye(128, dtype=np.float32)
    expand = np.repeat(
        np.eye(NUM_EXPERTS, dtype=np.float32) * np.float32(SCALING), RANK, axis=1
    ).astype(NPBF16)
    return dict(wt=wt, at=at, bt=bt, rt=rt, bias2=bias2, id2=id2, expand=expand)


def make_in_maps(x, W_base, b_base, W_router, A_stack, B_stack, T=1024, KT=32, OT=32):
    shared = prep_shared(W_base, b_base, W_router, A_stack, B_stack, KT, OT)
    xf = np.asarray(x, dtype=np.float32).reshape(-1, D_IN)
    in_maps = []
    for c in range(N_CORES):
        x_c = xf[c * T : (c + 1) * T]  # [T, D]
        xt = np.ascontiguousarray(
            x_c.reshape(T, KT, 128).transpose(2, 1, 0)
        ).astype(NPBF16)
        m = dict(shared)
        m["xt"] = xt
        in_maps.append(m)
    return in_maps


_NC_CACHE = {}


def _get_nc(T, KT, OT):
    key = (T, KT, OT)
    if key not in _NC_CACHE:
        _NC_CACHE[key] = build_nc(T, KT, OT)
    return _NC_CACHE[key]


def kernel(x, W_base, b_base, W_router, A_stack, B_stack):
    x = np.asarray(x, dtype=np.float32)
    orig_shape = x.shape
    N = x.reshape(-1, D_IN).shape[0]
    T = N // N_CORES
    KT = D_IN // 128
    OT = D_OUT // 128

    nc = _get_nc(T, KT, OT)
    in_maps = make_in_maps(x, W_base, b_base, W_router, A_stack, B_stack, T, KT, OT)

    res = run_bass_kernel_spmd(nc, in_maps, core_ids=list(range(N_CORES)))
    out = np.empty((N, D_OUT), dtype=np.float32)
    for c in range(N_CORES):
        outt = res.results[c]["outt"]  # [OT, 128, T] bf16
        out[c * T : (c + 1) * T] = (
            outt.astype(np.float32).transpose(2, 0, 1).reshape(T, D_OUT)
        )
    return out.reshape(orig_shape[:-1] + (D_OUT,))
